# revision 19
# baseline (speedup 1.0000x reference)
"""Batched ChebConv (K=3) Trainium2 kernel.

Strategy (dst-node sharding, 8 cores):
  - Nodes padded to 10240 = 80 windows x 128. Core c owns windows
    [10c, 10c+10) = nodes [1280c, 1280c+1280), all B=8 batches.
  - All batches ride in the free dim: gather rows are [512] f32 (2KB).
  - Propagation P(h)[col] += norm_e * h[row]:
      host sorts edges by destination window; per 128-edge chunk the
      vector engine builds a one-hot scatter matrix S[e, dst_local] =
      norm_e (iota-compare against dst_local, scaled by norm), and the
      PE accumulates psum[128 dst, 512] += S.T @ gathered[128 e, 512].
      Source rows are fetched with dma_gather (SWDGE indexed gather,
      int16 indices) from HBM.
  - Launch 1: Tx1 slices for all cores -> host assembles full Tx1.
    Launch 2: gathers from Tx1, Tx2 = 2*P(Tx1) - x, then the output
    epilogue out = x@W0 + Tx1@W1 + Tx2@W2 + bias via PE transposes
    (output written d-major; host untransposes).
"""

import os
import numpy as np

NC_CORES = 8
NPW = 128  # nodes per window


# ----------------------------------------------------------------------------
# host-side prep
# ----------------------------------------------------------------------------

def _prep_edges(edge_index, edge_attr, n_nodes, n_windows):
    """Sort edges by destination window; pad each window to CH chunks of 128.

    Returns (CH, src_pad[NW, CH*128] int16, dstl_pad[NW, CH*128] f32,
    norm_pad[NW, CH*128] f32).
    """
    row = edge_index[0].astype(np.int64)
    col = edge_index[1].astype(np.int64)
    ea = edge_attr.astype(np.float64)

    deg = np.zeros(n_nodes, np.float64)
    np.add.at(deg, row, ea)
    deg = deg.astype(np.float32)
    dis = np.where(deg > 0, 1.0 / np.sqrt(deg), 0.0).astype(np.float32)
    norm = -(dis[row] * edge_attr.astype(np.float32) * dis[col])

    # sort by (window, src): window grouping is required for the scatter;
    # src-sorting within a window makes the HBM gather near-sequential.
    w_of_edge = col // NPW
    order = np.lexsort((row, w_of_edge))
    cnt = np.bincount(w_of_edge, minlength=n_windows)
    ch = int(np.ceil(cnt.max() / 128))  # chunks per window
    slots = ch * 128

    src_pad = np.zeros((n_windows, slots), np.int16)
    dstl_pad = np.zeros((n_windows, slots), np.float32)
    norm_pad = np.zeros((n_windows, slots), np.float32)
    srt_row = row[order]
    srt_col = col[order]
    srt_norm = norm[order]
    pos = np.concatenate([[0], np.cumsum(cnt)])
    for w in range(n_windows):
        e0, e1 = int(pos[w]), int(pos[w + 1])
        k = e1 - e0
        src_pad[w, :k] = srt_row[e0:e1]
        dstl_pad[w, :k] = (srt_col[e0:e1] - w * NPW).astype(np.float32)
        norm_pad[w, :k] = srt_norm[e0:e1]
    return ch, src_pad, dstl_pad, norm_pad


def _wrap16(a):
    """Element i -> [i%16, i//16], replicated to 128 partitions."""
    n = a.shape[-1]
    w = a.reshape(*a.shape[:-1], n // 16, 16)
    w = np.swapaxes(w, -1, -2)  # [..., 16, n//16]
    return np.concatenate([w] * 8, axis=-2)  # [..., 128, n//16]


def _wrap128(a):
    """Element i -> [i%128, i//128]."""
    n = a.shape[-1]
    w = a.reshape(*a.shape[:-1], n // 128, 128)
    return np.swapaxes(w, -1, -2)


# ----------------------------------------------------------------------------
# device program
# ----------------------------------------------------------------------------

def _build_prog(ch, wpc, npad, bd, epilogue):
    """One SPMD program: per-core propagation over `wpc` windows of `ch`
    chunks; if `epilogue`, also Tx2 and the W-projection output stage."""
    from concourse import bacc, tile, library_config
    import concourse.mybir as mybir

    f32 = mybir.dt.float32
    i16 = mybir.dt.int16
    eq = mybir.AluOpType.is_equal
    mul = mybir.AluOpType.mult
    sub = mybir.AluOpType.subtract
    add = mybir.AluOpType.add

    GSEG = 8  # chunks per dma_gather call (1024 idxs; HW fails above ~1k)
    segs = [GSEG] * (ch // GSEG)
    if ch % GSEG:
        segs.append(ch % GSEG)
    nown = wpc * NPW  # nodes owned per core

    nc = bacc.Bacc(
        "TRN2", target_bir_lowering=False, debug=False, num_devices=NC_CORES
    )

    srcg = nc.dram_tensor("srcg", [npad, bd], f32, kind="ExternalInput")
    idx_d = nc.dram_tensor("idx", [wpc, 128, ch * 8], i16, kind="ExternalInput")
    dst_d = nc.dram_tensor("dstl", [wpc, 128, ch], f32, kind="ExternalInput")
    nrm_d = nc.dram_tensor("nrm", [wpc, 128, ch], f32, kind="ExternalInput")
    iota_d = nc.dram_tensor("iota", [128, 128], f32, kind="ExternalInput")
    if epilogue:
        ident_d = nc.dram_tensor("ident", [128, 128], f32, kind="ExternalInput")
        xown_d = nc.dram_tensor("xown", [nown, bd], f32, kind="ExternalInput")
        t1own_d = nc.dram_tensor("t1own", [nown, bd], f32, kind="ExternalInput")
        w_d = nc.dram_tensor("w", [3, 64, 64], f32, kind="ExternalInput")
        bias_d = nc.dram_tensor("bias", [64, 1], f32, kind="ExternalInput")
        outt_d = nc.dram_tensor("outt", [wpc, 64, 1024], f32, kind="ExternalOutput")
    else:
        tx1_d = nc.dram_tensor("tx1", [nown, bd], f32, kind="ExternalOutput")

    with tile.TileContext(nc) as tc:
        nc.gpsimd.load_library(library_config.mlp)
        with (
            tc.tile_pool(name="const", bufs=1) as constp,
            tc.tile_pool(name="gat", bufs=4) as gatp,
            tc.tile_pool(name="gatr", bufs=2) as gatrp,
            tc.tile_pool(name="meta", bufs=2) as metap,
            tc.tile_pool(name="oh", bufs=4) as ohp,
            tc.tile_pool(name="outp", bufs=2) as outp,
            tc.tile_pool(name="ps", bufs=2, space="PSUM") as psp,
            tc.tile_pool(name="tps", bufs=2, space="PSUM") as tpsp,
            tc.tile_pool(name="ops", bufs=1, space="PSUM") as opsp,
        ):
            iota_t = constp.tile([128, 128], f32, tag="iota")
            nc.sync.dma_start(iota_t[:], iota_d[:])
            if epilogue:
                ident_t = constp.tile([128, 128], f32, tag="ident")
                nc.sync.dma_start(ident_t[:], ident_d[:])
                w_t = constp.tile([64, 3, 64], f32, tag="w")
                nc.sync.dma_start(w_t[:], w_d.ap().rearrange("k d e -> d k e"))
                bias_t = constp.tile([64, 1], f32, tag="bias")
                nc.sync.dma_start(bias_t[:], bias_d[:])

            for w in range(wpc):
                idx_t = metap.tile([128, ch * 8], i16, tag="idx")
                nc.sync.dma_start(idx_t[:], idx_d[w])
                g_ts = []
                c0 = 0
                for seg in segs:
                    pool = gatp if seg == GSEG else gatrp
                    g_t = pool.tile(
                        [128, seg, bd], f32, tag="g" if seg == GSEG else "gr"
                    )
                    nc.gpsimd.dma_gather(
                        g_t[:],
                        srcg.ap(),
                        idx_t[:, c0 * 8 : (c0 + seg) * 8],
                        seg * 128,
                        seg * 128,
                        bd,
                    )
                    g_ts.append(g_t)
                    c0 += seg
                dst_t = metap.tile([128, ch], f32, tag="dst")
                nc.sync.dma_start(dst_t[:], dst_d[w])
                nrm_t = metap.tile([128, ch], f32, tag="nrm")
                nc.sync.dma_start(nrm_t[:], nrm_d[w])

                ps = psp.tile([128, bd], f32, tag="acc")
                for c in range(ch):
                    h, cc = divmod(c, GSEG)
                    s_t = ohp.tile([128, 128], f32, tag="s")
                    nc.vector.tensor_scalar(
                        s_t[:],
                        iota_t[:],
                        dst_t[:, c : c + 1],
                        nrm_t[:, c : c + 1],
                        op0=eq,
                        op1=mul,
                    )
                    nc.tensor.matmul(
                        ps[:],
                        s_t[:],
                        g_ts[h][:, cc, :],
                        start=(c == 0),
                        stop=(c == ch - 1),
                    )

                if not epilogue:
                    o_t = outp.tile([128, bd], f32, tag="o")
                    nc.vector.tensor_copy(o_t[:], ps[:])
                    nc.sync.dma_start(tx1_d[w * NPW : (w + 1) * NPW, :], o_t[:])
                else:
                    xw = outp.tile([128, bd], f32, tag="xw")
                    nc.sync.dma_start(xw[:], xown_d[w * NPW : (w + 1) * NPW, :])
                    t1w = outp.tile([128, bd], f32, tag="t1w")
                    nc.sync.dma_start(t1w[:], t1own_d[w * NPW : (w + 1) * NPW, :])
                    t2w = outp.tile([128, bd], f32, tag="t2w")
                    # Tx2 = 2*P(Tx1) - x
                    nc.vector.tensor_scalar(t2w[:], ps[:], 2.0, None, op0=mul)
                    nc.vector.tensor_tensor(t2w[:], t2w[:], xw[:], op=sub)

                    ops = opsp.tile([64, 1024], f32, tag="ot")
                    for b in range(8):
                        tps = tpsp.tile([64, 384], f32, tag="tp")
                        for k, src_t in enumerate((xw, t1w, t2w)):
                            nc.tensor.transpose(
                                tps[:, k * 128 : (k + 1) * 128],
                                src_t[:, b * 64 : (b + 1) * 64],
                                ident_t[:],
                            )
                        tsb = outp.tile([64, 384], f32, tag="tsb")
                        nc.vector.tensor_copy(tsb[:], tps[:])
                        for k in range(3):
                            nc.tensor.matmul(
                                ops[:, b * 128 : (b + 1) * 128],
                                w_t[:, k, :],
                                tsb[:, k * 128 : (k + 1) * 128],
                                start=(k == 0),
                                stop=(k == 2),
                            )
                    osb = outp.tile([64, 1024], f32, tag="osb")
                    nc.vector.tensor_scalar(osb[:], ops[:], bias_t[:, 0:1], None, op0=add)
                    nc.sync.dma_start(outt_d[w], osb[:])
    nc.compile()
    return nc


# ----------------------------------------------------------------------------
# entry point
# ----------------------------------------------------------------------------

LAST_EXEC_NS = []


def _launch(nc, in_maps, trace):
    from concourse.bass_utils import run_bass_kernel_spmd

    return run_bass_kernel_spmd(nc, in_maps, list(range(len(in_maps))), trace=trace)


def kernel(x, edge_index, edge_attr, W, bias):
    trace = bool(int(os.environ.get("CHEB_TRACE", "0")))

    B, N, D = x.shape
    bd = B * D
    nw = -(-N // NPW)  # windows for real nodes
    nw = -(-nw // NC_CORES) * NC_CORES  # pad to multiple of cores
    wpc = nw // NC_CORES
    npad = nw * NPW
    nown = wpc * NPW

    ch, src_pad, dstl_pad, norm_pad = _prep_edges(edge_index, edge_attr, N, nw)

    # gather source: node-major, all batches contiguous
    xg = np.zeros((npad, bd), np.float32)
    xg[:N] = np.ascontiguousarray(x.transpose(1, 0, 2)).reshape(N, bd)

    idx_all = _wrap16(src_pad)  # [nw, 128, ch*8]
    dst_all = _wrap128(dstl_pad)  # [nw, ch, 128] -> want [nw, 128, ch]
    nrm_all = _wrap128(norm_pad)

    iota = np.broadcast_to(np.arange(128, dtype=np.float32), (128, 128)).copy()
    ident = np.eye(128, dtype=np.float32)

    core_ids = list(range(NC_CORES))

    # ---- launch 1: Tx1 = P(x) ----
    prog1 = _build_prog(ch, wpc, npad, bd, epilogue=False)
    in_maps1 = []
    for c in core_ids:
        ws = slice(c * wpc, (c + 1) * wpc)
        in_maps1.append(
            {
                "srcg": xg,
                "idx": np.ascontiguousarray(idx_all[ws]),
                "dstl": np.ascontiguousarray(dst_all[ws]),
                "nrm": np.ascontiguousarray(nrm_all[ws]),
                "iota": iota,
            }
        )
    r1 = _launch(prog1, in_maps1, trace)
    tx1 = np.concatenate([r1.results[c]["tx1"] for c in core_ids], axis=0)

    # ---- launch 2: Tx2 + projection epilogue ----
    prog2 = _build_prog(ch, wpc, npad, bd, epilogue=True)
    in_maps2 = []
    for c in core_ids:
        ws = slice(c * wpc, (c + 1) * wpc)
        rs = slice(c * nown, (c + 1) * nown)
        in_maps2.append(
            {
                "srcg": tx1,
                "idx": np.ascontiguousarray(idx_all[ws]),
                "dstl": np.ascontiguousarray(dst_all[ws]),
                "nrm": np.ascontiguousarray(nrm_all[ws]),
                "iota": iota,
                "ident": ident,
                "xown": np.ascontiguousarray(xg[rs]),
                "t1own": np.ascontiguousarray(tx1[rs]),
                "w": W.astype(np.float32),
                "bias": bias.astype(np.float32).reshape(64, 1),
            }
        )
    r2 = _launch(prog2, in_maps2, trace)

    global LAST_EXEC_NS
    LAST_EXEC_NS = [r1.exec_time_ns, r2.exec_time_ns]

    # outt[w, e, b*128+nl] = out[b, core*1280 + w*128 + nl, e]
    out = np.empty((B, npad, 64), np.float32)
    for c in core_ids:
        ot = r2.results[c]["outt"].reshape(wpc, 64, 8, 128)
        # -> [b, w, nl, e]
        ot = ot.transpose(2, 0, 3, 1).reshape(B, nown, 64)
        out[:, c * nown : (c + 1) * nown, :] = ot
    return out[:, :N, :]
